# revision 25
# baseline (speedup 1.0000x reference)
"""Trainium2 Bass kernel for nn_Attention1 (dual-source cross-attention).

Reference, per (b,s) "batch row" bs in [0,32):
  k1,v1 = x @ W_qkv1.T (per head), k2,v2 = y @ W_qkv2.T,
  q = concat(x,y) @ W_qkv3.T,
  attn = softmax(scale * [q@k1.T | q@k2.T])          -> output #2
  out_heads = attn @ [v1;v2]                          [BS, H, N3, hd]
  # faithful torch transpose bug: [1,BS,H,N3,hd] -swap(1,2)-> [1,H,BS,N3,hd]
  # reshaped flat to (B,S,N3,C) then @ W_proj.T + b_proj -> output #1
The scrambled reshape means: chunk (bs,h) viewed as (49, 384) rows lands at
out[r = 4h + bs//8, 49j:49j+49, :] with j = bs%8.

Sharding: data-parallel over bs across 8 cores (4 bs each), weights
replicated. Device computes per core (keys-major attention so the exp'd
scores feed attn@v directly; softmax denominators via a ones matmul):
  - attn_t [4, H, 392(keys), 392(q)]  unnormalized exp(scale*scores)
  - dens   [4, H, 392(q)]             sum over keys of exp
  - po     [384, 4*392]               projected chunk rows, feature-major
Host: attn = attn_t.T / dens, scatter po into the final output, + b_proj.
"""

import numpy as np

B, S, N1, C = 8, 4, 196, 384
H, HD = 8, 48
BS = B * S            # 32
N3 = 2 * N1           # 392
NCORES = 8
BPC = BS // NCORES    # 4 bs per core
SCALE = float(HD) ** -0.5

_PROGRAM = None

# key tiles within a bs block: (start, size); {x:128,68 | y:128,68}
KT = [(0, 128), (128, 68), (196, 128), (324, 68)]
# exp bank-pairs group kt tiles with equal partition counts
EPAIRS = [((0, 2), 128), ((1, 3), 68)]
# eA column-block index for each kt (pair-contiguous layout)
EBLK = {0: 0, 2: 1, 1: 2, 3: 3}


def _build_program():
    import concourse.bass as bass
    import concourse.mybir as mybir
    import concourse.tile as tile

    f32 = mybir.dt.float32
    F32R = mybir.dt.float32r
    AF = mybir.ActivationFunctionType

    nc = bass.Bass("TRN2", target_bir_lowering=False, debug=False,
                   num_devices=NCORES)

    xyT = nc.dram_tensor("xyT", [C, BPC * N3], F32R, kind="ExternalInput").ap()
    wk = nc.dram_tensor("wk", [2, C, 512], F32R, kind="ExternalInput").ap()
    wv = nc.dram_tensor("wv", [2, C, 384], F32R, kind="ExternalInput").ap()
    wq = nc.dram_tensor("wq", [C, 512], F32R, kind="ExternalInput").ap()
    wpa = nc.dram_tensor("wpa", [48, 8 * C], F32R, kind="ExternalInput").ap()
    attn_t = nc.dram_tensor("attn_t", [BPC, H, 4, 128, N3], F32R,
                            kind="ExternalOutput").ap()
    dens = nc.dram_tensor("dens", [BPC, H, N3], F32R,
                          kind="ExternalOutput").ap()
    po = nc.dram_tensor("po", [C, BPC * N3], f32, kind="ExternalOutput").ap()

    with tile.TileContext(nc) as tc:
        with (
            tc.tile_pool(name="const", bufs=1) as const,
            tc.tile_pool(name="psS", bufs=3, space="PSUM") as ps_s,
            tc.tile_pool(name="psA", bufs=2, space="PSUM") as ps_a,
            tc.tile_pool(name="psV", bufs=2, space="PSUM") as ps_v,
            tc.tile_pool(name="psVd", bufs=1, space="PSUM") as ps_vd,
            tc.tile_pool(name="work", bufs=3) as work,
            tc.tile_pool(name="otp", bufs=2) as otp,
            tc.tile_pool(name="pop", bufs=3) as pop,
        ):
            # ---- persistent SBUF tensors -------------------------------
            xy_sb = [const.tile([128, BPC * N3], F32R, tag=f"xy{k}",
                                name=f"xy{k}") for k in range(3)]
            wk_sb = [[const.tile([128, 512], F32R, tag=f"wk{b}{k}",
                                 name=f"wk{b}{k}") for k in range(3)]
                     for b in range(2)]
            wv_sb = [[const.tile([128, 384], F32R, tag=f"wv{b}{k}",
                                 name=f"wv{b}{k}") for k in range(3)]
                     for b in range(2)]
            wq_sb = [const.tile([128, 512], F32R, tag=f"wq{k}",
                                name=f"wq{k}") for k in range(3)]
            wpa_sb = const.tile([48, 8 * C], F32R, tag="wpa", name="wpa")
            ones_sb = const.tile([1, 128], F32R, tag="ones", name="ones")
            ones_col = const.tile([128, 1], F32R, tag="onesc", name="onesc")
            kT = [const.tile([128, BPC * N3 + 60], F32R, tag=f"kT{t}",
                             name=f"kT{t}") for t in range(4)]
            qT = [const.tile([128, BPC * N3], F32R, tag=f"qT{t}",
                             name=f"qT{t}") for t in range(4)]
            v_sb = {}
            for l in range(BPC):
                for br in range(2):
                    for sub in range(2):
                        v_sb[(l, br, sub)] = const.tile(
                            [128, 384], F32R, tag=f"v{l}{br}{sub}",
                            name=f"v{l}{br}{sub}")

            # ---- load inputs -------------------------------------------
            for k in range(3):
                nc.sync.dma_start(xy_sb[k][:], xyT[k * 128:(k + 1) * 128, :])
                nc.sync.dma_start(wq_sb[k][:], wq[k * 128:(k + 1) * 128, :])
                for b in range(2):
                    nc.sync.dma_start(wk_sb[b][k][:],
                                      wk[b, k * 128:(k + 1) * 128, :])
                    nc.sync.dma_start(wv_sb[b][k][:],
                                      wv[b, k * 128:(k + 1) * 128, :])
            nc.sync.dma_start(wpa_sb[:], wpa[:, :])
            ones_f32 = const.tile([128, 128], f32, tag="onesf", name="onesf")
            nc.gpsimd.memset(ones_f32[:], 1.0)
            nc.vector.tensor_copy(ones_sb[0:1, :], ones_f32[0:1, :])
            nc.vector.tensor_copy(ones_col[:, 0:1], ones_f32[:, 0:1])
            for t in range(4):
                nc.vector.tensor_copy(kT[t][:, BPC * N3:BPC * N3 + 60],
                                      ones_f32[:, 0:60])

            # ---- k/q projections (feature-major, 64-padded head pairs) -
            # kT[t]: partition 64*(h%2)+d for head pair t=h//2;
            # col = l*392 + br*196 + tok.  qT[t]: col = l*392 + tok(xy)
            for t in range(4):
                for br in range(2):
                    for n in range(2):  # bs pairs
                        ps = ps_a.tile([128, 392], f32, tag="psA",
                                       name="psA")
                        for k in range(3):
                            rhs = xy_sb[k][:].rearrange(
                                "p (l tok) -> p l tok", tok=N3)[
                                :, 2 * n:2 * n + 2, br * N1:(br + 1) * N1]
                            nc.tensor.matmul(
                                ps[:, 0:392],
                                wk_sb[br][k][:, t * 128:(t + 1) * 128],
                                rhs, start=(k == 0), stop=(k == 2))
                        dst = kT[t][:, 0:BPC * N3].rearrange(
                            "p (l tok) -> p l tok", tok=N3)[
                            :, 2 * n:2 * n + 2, br * N1:(br + 1) * N1]
                        src = ps[:, 0:392].rearrange(
                            "p (l tok) -> p l tok", tok=N1)
                        nc.vector.tensor_copy(dst, src)
                for n in range(4):  # per bs
                    ps = ps_a.tile([128, 392], f32, tag="psA", name="psA")
                    for k in range(3):
                        nc.tensor.matmul(
                            ps[:, 0:392],
                            wq_sb[k][:, t * 128:(t + 1) * 128],
                            xy_sb[k][:, n * N3:(n + 1) * N3],
                            start=(k == 0), stop=(k == 2))
                    nc.vector.tensor_copy(qT[t][:, n * N3:(n + 1) * N3],
                                          ps[:, 0:392])

            # ---- v projections (token-major) ---------------------------
            for l in range(BPC):
                for br in range(2):
                    for sub in range(2):
                        tsz = 128 if sub == 0 else 68
                        tok0 = l * N3 + br * N1 + sub * 128
                        ps = ps_a.tile([128, 392], f32, tag="psA",
                                       name="psA")
                        for k in range(3):
                            nc.tensor.matmul(
                                ps[0:tsz, 0:384],
                                xy_sb[k][:, tok0:tok0 + tsz],
                                wv_sb[br][k][:],
                                start=(k == 0), stop=(k == 2))
                        vt = v_sb[(l, br, sub)]
                        nc.vector.tensor_copy(vt[0:tsz, :], ps[0:tsz, 0:384])

            # ---- attention (keys-major) --------------------------------
            for l in range(BPC):
                oT = otp.tile([48, H * N3], F32R, tag="oT", name="oT")
                dr_list = []
                for h in range(H):
                    tp, pb = h // 2, 64 * (h % 2)
                    q_ap = qT[tp][pb:pb + 48, l * N3:(l + 1) * N3]

                    # scoresT [keys, q] in bank pairs; exp -> eA
                    # eA col blocks in pair order: [kt0 | kt2 | kt1 | kt3]
                    eA = work.tile([128, 4 * N3], F32R, tag="eA",
                                   name="eA", bufs=4)
                    for kt in range(4):
                        k0, ksz = KT[kt]
                        sp = ps_s.tile([128, 392], f32, tag="psS",
                                       name="psS")
                        nc.tensor.matmul(
                            sp[:, :],
                            kT[tp][pb:pb + 48,
                                   l * N3 + k0:l * N3 + k0 + 128],
                            q_ap, start=True, stop=True)
                        nc.scalar.activation(
                            eA[:, kt * N3:(kt + 1) * N3], sp[:, :],
                            AF.Exp, scale=SCALE)

                    # attn@v accumulation (rows 0-47), then den (row 64)
                    av = ps_v.tile([48, 392], f32, tag="psV", name="psV")
                    avd = ps_vd.tile([1, 392], f32, tag="psVd", name="psVd")
                    for kt in range(4):
                        k0, ksz = KT[kt]
                        br, sub = kt // 2, kt % 2
                        nc.tensor.matmul(
                            av[0:48, :],
                            v_sb[(l, br, sub)][0:ksz, 48 * h:48 * h + 48],
                            eA[0:ksz, kt * N3:kt * N3 + N3],
                            start=(kt == 0), stop=(kt == 3))
                        nc.tensor.matmul(
                            avd[0:1, :], ones_col[0:ksz, 0:1],
                            eA[0:ksz, kt * N3:kt * N3 + N3],
                            start=(kt == 0), stop=(kt == 3))

                    # stream unnormalized exp out (kt-padded, host slices)
                    nc.sync.dma_start(
                        attn_t[l, h, :, :, :].rearrange(
                            "kt p c -> p kt c"),
                        eA[:, :].rearrange("p (kt c) -> p kt c", c=N3))

                    nc.vector.tensor_copy(oT[0:48, h * N3:(h + 1) * N3],
                                          av[0:48, :])
                    dr = otp.tile([1, N3], F32R, tag="denrow",
                                  name="denrow", bufs=4)
                    nc.vector.tensor_copy(dr[0:1, :], avd[0:1, :])
                    nc.sync.dma_start(dens[l, h, :], dr[0:1, :])
                    dr_list.append(dr)

                # ---- normalize out_T rows by 1/den ---------------------
                for h in range(H):
                    dr = dr_list[h]
                    with nc.allow_low_precision(reason="f32r feeds matmul"):
                        nc.vector.reciprocal(dr[0:1, :], dr[0:1, :])
                    bc = ps_a.tile([128, 392], f32, tag="psA", name="psA")
                    nc.tensor.matmul(bc[0:48, :], ones_sb[0:1, 0:48],
                                     dr[0:1, :], start=True, stop=True)
                    nc.vector.tensor_mul(oT[0:48, h * N3:(h + 1) * N3],
                                         oT[0:48, h * N3:(h + 1) * N3],
                                         bc[0:48, :])

                # ---- output projection over scrambled chunks -----------
                oT_v = oT[0:48, :].rearrange("p (h t e) -> p h t e",
                                             t=49, e=8)
                for m in range(3):
                    pp = ps_a.tile([128, 392], f32, tag="psA", name="psA")
                    for a in range(8):
                        rhs = oT_v[:, :, :, a]
                        nc.tensor.matmul(
                            pp[:],
                            wpa_sb[0:48,
                                   a * C + m * 128:a * C + (m + 1) * 128],
                            rhs, start=(a == 0), stop=(a == 7))
                    posb = pop.tile([128, 392], f32, tag="po", name="po")
                    nc.vector.tensor_copy(posb[:], pp[:])
                    nc.sync.dma_start(
                        po[m * 128:(m + 1) * 128, l * N3:(l + 1) * N3],
                        posb[:])

    return nc


def _split_waits(nc, mybir, maxw=1):
    """Split instructions with >maxw sem-waits (walrus TPB_CTRL limit)."""
    for fn in nc.m.functions:
        for bb in fn.blocks:
            new_list = []
            for inst in bb.instructions:
                w = inst.sync_info.on_wait if inst.sync_info else None
                if w and len(w) > maxw:
                    waits = list(w)
                    k = 0
                    while len(waits) - k > maxw:
                        chunk = waits[k:k + maxw]
                        k += maxw
                        nd = mybir.InstDrain(
                            name=f"{inst.name}-wsplit-{k}",
                            ins=[], outs=[],
                            sync_info=mybir.SyncInfo(on_wait=chunk,
                                                     on_update=[]),
                        )
                        nd.engine = inst.engine
                        new_list.append(nd)
                    inst.sync_info.on_wait = waits[k:]
                new_list.append(inst)
            bb.instructions[:] = new_list


def _host_prep(x, y, W_qkv1, W_qkv2, W_qkv3, W_proj):
    """Build per-core input maps (numpy, all float32)."""
    x = np.asarray(x, dtype=np.float32).reshape(BS, N1, C)
    y = np.asarray(y, dtype=np.float32).reshape(BS, N1, C)
    W1 = np.asarray(W_qkv1, dtype=np.float32)
    W2 = np.asarray(W_qkv2, dtype=np.float32)
    W3 = np.asarray(W_qkv3, dtype=np.float32)
    Wp = np.asarray(W_proj, dtype=np.float32)

    def pad_heads(w):  # w [384, C] head-major rows -> [C, 512] padded cols
        out = np.zeros((C, 512), dtype=np.float32)
        for h in range(H):
            out[:, 64 * h:64 * h + 48] = w[48 * h:48 * h + 48, :].T
        return out

    wk_np = np.stack([pad_heads(W1[:C]), pad_heads(W2[:C])])
    wv_np = np.stack([np.ascontiguousarray(W1[C:].T),
                      np.ascontiguousarray(W2[C:].T)])
    wq_np = pad_heads(W3)
    # wpa[d, a*384 + c2] = W_proj[c2, 48a + d]
    wpa_np = np.ascontiguousarray(
        Wp.T.reshape(8, 48, C).transpose(1, 0, 2).reshape(48, 8 * C))

    in_maps = []
    for c in range(NCORES):
        xs = x[BPC * c:BPC * (c + 1)]          # [4, 196, C]
        ys = y[BPC * c:BPC * (c + 1)]
        xy = np.concatenate([xs, ys], axis=1)  # [4, 392, C]
        xyT_np = np.ascontiguousarray(xy.reshape(BPC * N3, C).T)
        in_maps.append({
            "xyT": xyT_np, "wk": wk_np, "wv": wv_np, "wq": wq_np,
            "wpa": wpa_np,
        })
    return in_maps


def _host_assemble(results, b_proj):
    b_proj = np.asarray(b_proj, dtype=np.float32)
    att_parts = []
    for res in results:
        ep = res["attn_t"]                         # [4, H, 4, 128, q] padded
        exp_t = np.concatenate(
            [ep[:, :, 0, 0:128], ep[:, :, 1, 0:68],
             ep[:, :, 2, 0:128], ep[:, :, 3, 0:68]], axis=2)
        den = res["dens"]                          # [4, H, q]
        att_parts.append(exp_t.transpose(0, 1, 3, 2) / den[..., :, None])
    attn_full = np.concatenate(att_parts, axis=0)  # [BS, H, q, keys]
    attn_full = attn_full.reshape(B, S, H, N3, N3).astype(np.float32)

    out_full = np.empty((BS, N3, C), dtype=np.float32)
    for c in range(NCORES):
        blk = results[c]["po"].T.reshape(BPC, 8, 49, C)
        # po col = l*392 + h*49 + t ; chunk (bs,h) -> row 4h + bs//8,
        # col block j = bs%8
        for l in range(BPC):
            bs = BPC * c + l
            for h in range(H):
                r_ = 4 * h + bs // 8
                j = bs % 8
                out_full[r_, 49 * j:49 * (j + 1), :] = blk[l, h]
    out_full = out_full + b_proj
    return out_full.reshape(B, S, N3, C), attn_full


def kernel(x, y, W_qkv1, W_qkv2, W_qkv3, W_proj, b_proj):
    global _PROGRAM
    from concourse.bass_utils import run_bass_kernel_spmd

    if _PROGRAM is None:
        import concourse.mybir as mybir
        _PROGRAM = _build_program()
        _split_waits(_PROGRAM, mybir)
    in_maps = _host_prep(x, y, W_qkv1, W_qkv2, W_qkv3, W_proj)
    res = run_bass_kernel_spmd(_PROGRAM, in_maps, list(range(NCORES)))
    return _host_assemble(res.results, b_proj)


# revision 26
# speedup vs baseline: 1.2309x; 1.2309x over previous
"""Trainium2 Bass kernel for nn_Attention1 (dual-source cross-attention).

Reference, per (b,s) "batch row" bs in [0,32):
  k1,v1 = x @ W_qkv1.T (per head), k2,v2 = y @ W_qkv2.T,
  q = concat(x,y) @ W_qkv3.T,
  attn = softmax(scale * [q@k1.T | q@k2.T])          -> output #2
  out_heads = attn @ [v1;v2]                          [BS, H, N3, hd]
  # faithful torch transpose bug: [1,BS,H,N3,hd] -swap(1,2)-> [1,H,BS,N3,hd]
  # reshaped flat to (B,S,N3,C) then @ W_proj.T + b_proj -> output #1
The scrambled reshape means: chunk (bs,h) viewed as (49, 384) rows lands at
out[r = 4h + bs//8, 49j:49j+49, :] with j = bs%8.

Sharding: data-parallel over bs across 8 cores (4 bs each), weights
replicated. Device computes per core (keys-major attention so the exp'd
scores feed attn@v directly; softmax denominators via a ones matmul):
  - attn_t [4, H, 392(keys), 392(q)]  unnormalized exp(scale*scores)
  - dens   [4, H, 392(q)]             sum over keys of exp
  - po     [384, 4*392]               projected chunk rows, feature-major
Host: attn = attn_t.T / dens, scatter po into the final output, + b_proj.
"""

import numpy as np

B, S, N1, C = 8, 4, 196, 384
H, HD = 8, 48
BS = B * S            # 32
N3 = 2 * N1           # 392
NCORES = 8
BPC = BS // NCORES    # 4 bs per core
SCALE = float(HD) ** -0.5

_PROGRAM = None

# key tiles within a bs block: (start, size); {x:128,68 | y:128,68}
KT = [(0, 128), (128, 68), (196, 128), (324, 68)]
# exp bank-pairs group kt tiles with equal partition counts
EPAIRS = [((0, 2), 128), ((1, 3), 68)]
# eA column-block index for each kt (pair-contiguous layout)
EBLK = {0: 0, 2: 1, 1: 2, 3: 3}


def _build_program():
    import concourse.bass as bass
    import concourse.mybir as mybir
    import concourse.tile as tile

    f32 = mybir.dt.float32
    F32R = mybir.dt.float32r
    AF = mybir.ActivationFunctionType

    nc = bass.Bass("TRN2", target_bir_lowering=False, debug=False,
                   num_devices=NCORES)

    xyT = nc.dram_tensor("xyT", [C, BPC * N3], F32R, kind="ExternalInput").ap()
    wk = nc.dram_tensor("wk", [2, C, 512], F32R, kind="ExternalInput").ap()
    wv = nc.dram_tensor("wv", [2, C, 384], F32R, kind="ExternalInput").ap()
    wq = nc.dram_tensor("wq", [C, 512], F32R, kind="ExternalInput").ap()
    wpa = nc.dram_tensor("wpa", [48, 8 * C], F32R, kind="ExternalInput").ap()
    attn_t = nc.dram_tensor("attn_t", [BPC, H, N3, N3], F32R,
                            kind="ExternalOutput").ap()
    dens = nc.dram_tensor("dens", [BPC, H, N3], F32R,
                          kind="ExternalOutput").ap()
    po = nc.dram_tensor("po", [C, BPC * N3], f32, kind="ExternalOutput").ap()

    with tile.TileContext(nc) as tc:
        with (
            tc.tile_pool(name="const", bufs=1) as const,
            tc.tile_pool(name="psS", bufs=3, space="PSUM") as ps_s,
            tc.tile_pool(name="psA", bufs=2, space="PSUM") as ps_a,
            tc.tile_pool(name="psV", bufs=2, space="PSUM") as ps_v,
            tc.tile_pool(name="psVd", bufs=1, space="PSUM") as ps_vd,
            tc.tile_pool(name="work", bufs=3) as work,
            tc.tile_pool(name="otp", bufs=2) as otp,
            tc.tile_pool(name="pop", bufs=3) as pop,
        ):
            # ---- persistent SBUF tensors -------------------------------
            xy_sb = [const.tile([128, BPC * N3], F32R, tag=f"xy{k}",
                                name=f"xy{k}") for k in range(3)]
            wk_sb = [[const.tile([128, 512], F32R, tag=f"wk{b}{k}",
                                 name=f"wk{b}{k}") for k in range(3)]
                     for b in range(2)]
            wv_sb = [[const.tile([128, 384], F32R, tag=f"wv{b}{k}",
                                 name=f"wv{b}{k}") for k in range(3)]
                     for b in range(2)]
            wq_sb = [const.tile([128, 512], F32R, tag=f"wq{k}",
                                name=f"wq{k}") for k in range(3)]
            wpa_sb = const.tile([48, 8 * C], F32R, tag="wpa", name="wpa")
            ones_sb = const.tile([1, 128], F32R, tag="ones", name="ones")
            ones_col = const.tile([128, 1], F32R, tag="onesc", name="onesc")
            kT = [const.tile([128, BPC * N3 + 60], F32R, tag=f"kT{t}",
                             name=f"kT{t}") for t in range(4)]
            qT = [const.tile([128, BPC * N3], F32R, tag=f"qT{t}",
                             name=f"qT{t}") for t in range(4)]
            v_sb = {}
            for l in range(BPC):
                for br in range(2):
                    for sub in range(2):
                        v_sb[(l, br, sub)] = const.tile(
                            [128, 384], F32R, tag=f"v{l}{br}{sub}",
                            name=f"v{l}{br}{sub}")

            # ---- load inputs -------------------------------------------
            for k in range(3):
                nc.sync.dma_start(xy_sb[k][:], xyT[k * 128:(k + 1) * 128, :])
                nc.sync.dma_start(wq_sb[k][:], wq[k * 128:(k + 1) * 128, :])
                for b in range(2):
                    nc.sync.dma_start(wk_sb[b][k][:],
                                      wk[b, k * 128:(k + 1) * 128, :])
                    nc.sync.dma_start(wv_sb[b][k][:],
                                      wv[b, k * 128:(k + 1) * 128, :])
            nc.sync.dma_start(wpa_sb[:], wpa[:, :])
            ones_f32 = const.tile([128, 128], f32, tag="onesf", name="onesf")
            nc.gpsimd.memset(ones_f32[:], 1.0)
            nc.vector.tensor_copy(ones_sb[0:1, :], ones_f32[0:1, :])
            nc.vector.tensor_copy(ones_col[:, 0:1], ones_f32[:, 0:1])
            for t in range(4):
                nc.vector.tensor_copy(kT[t][:, BPC * N3:BPC * N3 + 60],
                                      ones_f32[:, 0:60])

            # ---- k/q projections (feature-major, 64-padded head pairs) -
            # kT[t]: partition 64*(h%2)+d for head pair t=h//2;
            # col = l*392 + br*196 + tok.  qT[t]: col = l*392 + tok(xy)
            for t in range(4):
                for br in range(2):
                    for n in range(2):  # bs pairs
                        ps = ps_a.tile([128, 392], f32, tag="psA",
                                       name="psA")
                        for k in range(3):
                            rhs = xy_sb[k][:].rearrange(
                                "p (l tok) -> p l tok", tok=N3)[
                                :, 2 * n:2 * n + 2, br * N1:(br + 1) * N1]
                            nc.tensor.matmul(
                                ps[:, 0:392],
                                wk_sb[br][k][:, t * 128:(t + 1) * 128],
                                rhs, start=(k == 0), stop=(k == 2))
                        dst = kT[t][:, 0:BPC * N3].rearrange(
                            "p (l tok) -> p l tok", tok=N3)[
                            :, 2 * n:2 * n + 2, br * N1:(br + 1) * N1]
                        src = ps[:, 0:392].rearrange(
                            "p (l tok) -> p l tok", tok=N1)
                        nc.vector.tensor_copy(dst, src)
                for n in range(4):  # per bs
                    ps = ps_a.tile([128, 392], f32, tag="psA", name="psA")
                    for k in range(3):
                        nc.tensor.matmul(
                            ps[:, 0:392],
                            wq_sb[k][:, t * 128:(t + 1) * 128],
                            xy_sb[k][:, n * N3:(n + 1) * N3],
                            start=(k == 0), stop=(k == 2))
                    nc.vector.tensor_copy(qT[t][:, n * N3:(n + 1) * N3],
                                          ps[:, 0:392])

            # ---- v projections (token-major) ---------------------------
            for l in range(BPC):
                for br in range(2):
                    for sub in range(2):
                        tsz = 128 if sub == 0 else 68
                        tok0 = l * N3 + br * N1 + sub * 128
                        ps = ps_a.tile([128, 392], f32, tag="psA",
                                       name="psA")
                        for k in range(3):
                            nc.tensor.matmul(
                                ps[0:tsz, 0:384],
                                xy_sb[k][:, tok0:tok0 + tsz],
                                wv_sb[br][k][:],
                                start=(k == 0), stop=(k == 2))
                        vt = v_sb[(l, br, sub)]
                        nc.vector.tensor_copy(vt[0:tsz, :], ps[0:tsz, 0:384])

            # ---- attention (keys-major) --------------------------------
            for l in range(BPC):
                oT = otp.tile([48, H * N3], F32R, tag="oT", name="oT")
                dr_list = []
                for h in range(H):
                    tp, pb = h // 2, 64 * (h % 2)
                    q_ap = qT[tp][pb:pb + 48, l * N3:(l + 1) * N3]

                    # scoresT [keys, q] in bank pairs; exp -> eA
                    # eA col blocks in pair order: [kt0 | kt2 | kt1 | kt3]
                    e_tiles = []
                    for kt in range(4):
                        k0, ksz = KT[kt]
                        sp = ps_s.tile([128, 392], f32, tag="psS",
                                       name="psS")
                        nc.tensor.matmul(
                            sp[0:ksz, :],
                            kT[tp][pb:pb + 48,
                                   l * N3 + k0:l * N3 + k0 + ksz],
                            q_ap, start=True, stop=True)
                        e = work.tile([128, N3], F32R, tag="expT",
                                      name="expT", bufs=8)
                        nc.scalar.activation(e[0:ksz, :], sp[0:ksz, :],
                                             AF.Exp, scale=SCALE)
                        e_tiles.append(e)

                    # attn@v accumulation (rows 0-47), then den (row 64)
                    av = ps_v.tile([48, 392], f32, tag="psV", name="psV")
                    avd = ps_vd.tile([1, 392], f32, tag="psVd", name="psVd")
                    for kt in range(4):
                        k0, ksz = KT[kt]
                        br, sub = kt // 2, kt % 2
                        nc.tensor.matmul(
                            av[0:48, :],
                            v_sb[(l, br, sub)][0:ksz, 48 * h:48 * h + 48],
                            e_tiles[kt][0:ksz, :],
                            start=(kt == 0), stop=(kt == 3))
                        nc.tensor.matmul(
                            avd[0:1, :], ones_col[0:ksz, 0:1],
                            e_tiles[kt][0:ksz, :],
                            start=(kt == 0), stop=(kt == 3))

                    # stream unnormalized exp out
                    for kt in range(4):
                        k0, ksz = KT[kt]
                        nc.sync.dma_start(
                            attn_t[l, h, k0:k0 + ksz, :],
                            e_tiles[kt][0:ksz, :])

                    nc.vector.tensor_copy(oT[0:48, h * N3:(h + 1) * N3],
                                          av[0:48, :])
                    dr = otp.tile([1, N3], F32R, tag="denrow",
                                  name="denrow", bufs=4)
                    nc.vector.tensor_copy(dr[0:1, :], avd[0:1, :])
                    nc.sync.dma_start(dens[l, h, :], dr[0:1, :])
                    dr_list.append(dr)

                # ---- normalize out_T rows by 1/den ---------------------
                for h in range(H):
                    dr = dr_list[h]
                    with nc.allow_low_precision(reason="f32r feeds matmul"):
                        nc.vector.reciprocal(dr[0:1, :], dr[0:1, :])
                    bc = ps_a.tile([128, 392], f32, tag="psA", name="psA")
                    nc.tensor.matmul(bc[0:48, :], ones_sb[0:1, 0:48],
                                     dr[0:1, :], start=True, stop=True)
                    nc.vector.tensor_mul(oT[0:48, h * N3:(h + 1) * N3],
                                         oT[0:48, h * N3:(h + 1) * N3],
                                         bc[0:48, :])

                # ---- output projection over scrambled chunks -----------
                oT_v = oT[0:48, :].rearrange("p (h t e) -> p h t e",
                                             t=49, e=8)
                for m in range(3):
                    pp = ps_a.tile([128, 392], f32, tag="psA", name="psA")
                    for a in range(8):
                        rhs = oT_v[:, :, :, a]
                        nc.tensor.matmul(
                            pp[:],
                            wpa_sb[0:48,
                                   a * C + m * 128:a * C + (m + 1) * 128],
                            rhs, start=(a == 0), stop=(a == 7))
                    posb = pop.tile([128, 392], f32, tag="po", name="po")
                    nc.vector.tensor_copy(posb[:], pp[:])
                    nc.sync.dma_start(
                        po[m * 128:(m + 1) * 128, l * N3:(l + 1) * N3],
                        posb[:])

    return nc


def _split_waits(nc, mybir, maxw=1):
    """Split instructions with >maxw sem-waits (walrus TPB_CTRL limit)."""
    for fn in nc.m.functions:
        for bb in fn.blocks:
            new_list = []
            for inst in bb.instructions:
                w = inst.sync_info.on_wait if inst.sync_info else None
                if w and len(w) > maxw:
                    waits = list(w)
                    k = 0
                    while len(waits) - k > maxw:
                        chunk = waits[k:k + maxw]
                        k += maxw
                        nd = mybir.InstDrain(
                            name=f"{inst.name}-wsplit-{k}",
                            ins=[], outs=[],
                            sync_info=mybir.SyncInfo(on_wait=chunk,
                                                     on_update=[]),
                        )
                        nd.engine = inst.engine
                        new_list.append(nd)
                    inst.sync_info.on_wait = waits[k:]
                new_list.append(inst)
            bb.instructions[:] = new_list


def _host_prep(x, y, W_qkv1, W_qkv2, W_qkv3, W_proj):
    """Build per-core input maps (numpy, all float32)."""
    x = np.asarray(x, dtype=np.float32).reshape(BS, N1, C)
    y = np.asarray(y, dtype=np.float32).reshape(BS, N1, C)
    W1 = np.asarray(W_qkv1, dtype=np.float32)
    W2 = np.asarray(W_qkv2, dtype=np.float32)
    W3 = np.asarray(W_qkv3, dtype=np.float32)
    Wp = np.asarray(W_proj, dtype=np.float32)

    def pad_heads(w):  # w [384, C] head-major rows -> [C, 512] padded cols
        out = np.zeros((C, 512), dtype=np.float32)
        for h in range(H):
            out[:, 64 * h:64 * h + 48] = w[48 * h:48 * h + 48, :].T
        return out

    wk_np = np.stack([pad_heads(W1[:C]), pad_heads(W2[:C])])
    wv_np = np.stack([np.ascontiguousarray(W1[C:].T),
                      np.ascontiguousarray(W2[C:].T)])
    wq_np = pad_heads(W3)
    # wpa[d, a*384 + c2] = W_proj[c2, 48a + d]
    wpa_np = np.ascontiguousarray(
        Wp.T.reshape(8, 48, C).transpose(1, 0, 2).reshape(48, 8 * C))

    in_maps = []
    for c in range(NCORES):
        xs = x[BPC * c:BPC * (c + 1)]          # [4, 196, C]
        ys = y[BPC * c:BPC * (c + 1)]
        xy = np.concatenate([xs, ys], axis=1)  # [4, 392, C]
        xyT_np = np.ascontiguousarray(xy.reshape(BPC * N3, C).T)
        in_maps.append({
            "xyT": xyT_np, "wk": wk_np, "wv": wv_np, "wq": wq_np,
            "wpa": wpa_np,
        })
    return in_maps


def _host_assemble(results, b_proj):
    b_proj = np.asarray(b_proj, dtype=np.float32)
    att_parts = []
    for res in results:
        exp_t = res["attn_t"]                      # [4, H, keys, q]
        den = res["dens"]                          # [4, H, q]
        att_parts.append(exp_t.transpose(0, 1, 3, 2) / den[..., :, None])
    attn_full = np.concatenate(att_parts, axis=0)  # [BS, H, q, keys]
    attn_full = attn_full.reshape(B, S, H, N3, N3).astype(np.float32)

    out_full = np.empty((BS, N3, C), dtype=np.float32)
    for c in range(NCORES):
        blk = results[c]["po"].T.reshape(BPC, 8, 49, C)
        # po col = l*392 + h*49 + t ; chunk (bs,h) -> row 4h + bs//8,
        # col block j = bs%8
        for l in range(BPC):
            bs = BPC * c + l
            for h in range(H):
                r_ = 4 * h + bs // 8
                j = bs % 8
                out_full[r_, 49 * j:49 * (j + 1), :] = blk[l, h]
    out_full = out_full + b_proj
    return out_full.reshape(B, S, N3, C), attn_full


def kernel(x, y, W_qkv1, W_qkv2, W_qkv3, W_proj, b_proj):
    global _PROGRAM
    from concourse.bass_utils import run_bass_kernel_spmd

    if _PROGRAM is None:
        import concourse.mybir as mybir
        _PROGRAM = _build_program()
        _split_waits(_PROGRAM, mybir)
    in_maps = _host_prep(x, y, W_qkv1, W_qkv2, W_qkv3, W_proj)
    res = run_bass_kernel_spmd(_PROGRAM, in_maps, list(range(NCORES)))
    return _host_assemble(res.results, b_proj)
